# revision 21
# baseline (speedup 1.0000x reference)
"""MoE expert-pool kernel for 8 TRN2 NeuronCores (expert-parallel).

Strategy:
  - E=8 experts, one expert per core. Token routing (gather by
    expert_indices) is done on the host inside kernel(); each core
    receives only the tokens assigned to its expert, padded to a
    common capacity C (SPMD requires one NEFF / uniform shapes).
  - Everything on-device is laid out transposed (xT/hT/yT have the
    feature axis on partitions, tokens on the free axis) so both
    weight matrices stream in their natural layout as matmul lhsT
    and both biases are per-partition scalars for ACT/DVE.
  - Compute in bf16 (fp32 PSUM accumulation): hT = gelu(w1^T x + b1),
    yT = w2^T hT + b2. Host scatter-adds per-slot outputs (fp32).
"""

import numpy as np

_REPO = "/opt/trn_rl_repo"

_D = 1024  # d_model
_F = 4096  # d_ff
_P = 128   # partitions
_KD = _D // _P   # 8 contraction tiles along D
_KF = _F // _P   # 32 contraction tiles along F
_NB = 512        # token block = one fp32 PSUM bank

_NCORES = 8

_cache = {}
LAST_RESULT = None


def _ensure_path():
    import sys
    if _REPO not in sys.path:
        sys.path.insert(0, _REPO)


def _ensure_axon_hooks():
    """The container's `antenv` stub lacks `axon_hooks`, which
    bass_utils imports unconditionally on the traced (BASS_TRACE) axon
    path. Provide the missing get/set registry and register the NTFF
    ctypes hook the boot shim would have installed."""
    try:
        import antenv.axon_hooks  # noqa: F401
        return
    except ImportError:
        pass
    import sys
    import types
    mod = types.ModuleType("antenv.axon_hooks")
    mod._hook = None

    def set_axon_ntff_profile_hook(h):
        mod._hook = h

    def get_axon_ntff_profile_hook():
        return mod._hook

    mod.set_axon_ntff_profile_hook = set_axon_ntff_profile_hook
    mod.get_axon_ntff_profile_hook = get_axon_ntff_profile_hook
    sys.modules["antenv.axon_hooks"] = mod
    try:
        import antenv
        antenv.axon_hooks = mod
    except ImportError:
        pass
    try:
        from trn_agent_boot.trn_boot import _ntff_profile_via_ctypes
        hook = _ntff_profile_via_ctypes("/opt/axon/libaxon_pjrt.so")
        if hook is not None:
            mod._hook = hook
    except Exception:
        pass


def _build(C):
    _ensure_path()
    from concourse import bacc, mybir
    from concourse.tile import TileContext

    dt = mybir.dt
    AF = mybir.ActivationFunctionType

    # Bacc (not plain Bass): its compile() pass splits multi-sem waits
    # into event-semaphore instructions (TRN2 allows 1 wait/instruction).
    nc = bacc.Bacc("TRN2", target_bir_lowering=False, debug=False)
    xT = nc.declare_dram_parameter("xT", [_D, C], dt.bfloat16, isOutput=False)
    w1 = nc.declare_dram_parameter("w1", [_D, _F], dt.bfloat16, isOutput=False)
    w2 = nc.declare_dram_parameter("w2", [_F, _D], dt.bfloat16, isOutput=False)
    bia = nc.declare_dram_parameter("bias", [_P, _KF + _KD], dt.float32,
                                    isOutput=False)
    yT = nc.declare_dram_parameter("yT", [_D, C], dt.float32, isOutput=True)

    # Equal-ish token blocks of <=512 (one fp32 PSUM bank). Equal sizes
    # beat [512, 512, remainder]: per-column PE cost is flat above
    # N~128, so a tiny tail block wastes LDWEIGHTS-bound issue slots.
    nblk = -(-C // _NB)
    base = C // nblk // 8 * 8
    sizes = [base] * nblk
    extra = C - base * nblk
    i = 0
    while extra > 0:
        step = min(8, extra)
        sizes[i % nblk] += step
        extra -= step
        i += 1
    blocks = []
    s = 0
    for nb in sizes:
        blocks.append((s, nb))
        s += nb
    assert s == C

    nbmax = max(nb for _, nb in blocks)
    # SBUF layouts are DMA-chunk-major so every weight/activation DMA
    # writes one contiguous slice (exact dependency footprints, one DMA
    # instruction per chunk — DMA *issue* on the Sync engine costs
    # ~650 ns each and all transfers ride one 16-engine striped queue,
    # so fewer+bigger instructions shorten the PE ramp):
    #   xs : [block][k][nb_b]           (block-major)
    #   w1s: [fchunk of 512][k][512]    (8 chunks of 1 MB)
    #   w2s: [fgroup of 8 tiles][f%8][D] (4 groups of 2 MB)
    xbase = []
    o = 0
    for (_, nb) in blocks:
        xbase.append(o)
        o += _KD * nb
    W1C = 8            # w1 f-chunks
    W1CW = _F // W1C   # 512 cols per chunk
    W2G = 4            # w2 f-tile groups
    W2GW = _KF // W2G  # 8 f-tiles per group

    xTv = xT.rearrange("(k p) c -> p k c", p=_P)
    w1v = w1.rearrange("(k p) f -> p k f", p=_P)
    w2v = w2.rearrange("(f p) d -> p f d", p=_P)

    with TileContext(nc) as tc:
        with (
            tc.tile_pool(name="persist", bufs=1) as pers,
            tc.tile_pool(name="hpool", bufs=1) as hp,
            tc.tile_pool(name="ypool", bufs=3) as yp,
            tc.tile_pool(name="ph", bufs=4, space="PSUM") as php,
            tc.tile_pool(name="py", bufs=4, space="PSUM") as pyp,
        ):
            xs = pers.tile([_P, _KD * C], dt.bfloat16, name="xs")
            w1s = pers.tile([_P, _KD * _F], dt.bfloat16, name="w1s")
            w2s = pers.tile([_P, _KF * _D], dt.bfloat16, name="w2s")
            bs = pers.tile([_P, _KF + _KD], dt.float32, name="bs")

            def dma_xs(b):
                sb, nb = blocks[b]
                nc.sync.dma_start(
                    out=xs[:, xbase[b]: xbase[b] + _KD * nb].rearrange(
                        "p (k c) -> p k c", k=_KD),
                    in_=xTv[:, :, sb:sb + nb])

            # Issue order = arrival order (single striped queue). xs
            # block 0 lands first (one 0.7 MB DMA), then w1 chunk 0
            # per-k so the first k-accumulation starts after ~0.9 MB;
            # the bias is only needed by the first gelu (~17 us in).
            dma_xs(0)
            for k in range(_KD):
                nc.sync.dma_start(
                    out=w1s[:, k * W1CW: (k + 1) * W1CW],
                    in_=w1v[:, k, :W1CW])
            nc.sync.dma_start(out=bs[:, :], in_=bia[:, :])
            for cch in range(1, W1C):
                nc.sync.dma_start(
                    out=w1s[:, cch * _KD * W1CW: (cch + 1) * _KD * W1CW]
                    .rearrange("p (k f) -> p k f", k=_KD),
                    in_=w1v[:, :, cch * W1CW:(cch + 1) * W1CW])
            for b in range(1, len(blocks)):
                dma_xs(b)
            for g in range(W2G):
                nc.sync.dma_start(
                    out=w2s[:, g * W2GW * _D: (g + 1) * W2GW * _D]
                    .rearrange("p (f d) -> p f d", f=W2GW),
                    in_=w2v[:, g * W2GW:(g + 1) * W2GW, :])
            for bi, (s0, nb) in enumerate(blocks):
                hts = hp.tile([_P, _KF * nbmax], dt.bfloat16,
                              name="hts", tag="hts")
                tpc = W1CW // _P   # f-tiles per w1 chunk
                for f in range(_KF):
                    ph = php.tile([_P, _NB], dt.float32, name="ph", tag="ph")
                    cch, fi = f // tpc, f % tpc
                    for k in range(_KD):
                        nc.tensor.matmul(
                            ph[:, :nb],
                            lhsT=w1s[:, cch * _KD * W1CW + k * W1CW + fi * _P:
                                     cch * _KD * W1CW + k * W1CW + (fi + 1) * _P],
                            rhs=xs[:, xbase[bi] + k * nb: xbase[bi] + (k + 1) * nb],
                            start=(k == 0), stop=(k == _KD - 1))
                    nc.scalar.activation(
                        hts[:, f * nbmax: f * nbmax + nb], ph[:, :nb],
                        AF.Gelu, bias=bs[:, f:f + 1])
                for d in range(_KD):
                    py = pyp.tile([_P, _NB], dt.float32, name="py", tag="py")
                    for f in range(_KF):
                        g, fj = f // W2GW, f % W2GW
                        nc.tensor.matmul(
                            py[:, :nb],
                            lhsT=w2s[:, g * W2GW * _D + fj * _D + d * _P:
                                     g * W2GW * _D + fj * _D + (d + 1) * _P],
                            rhs=hts[:, f * nbmax: f * nbmax + nb],
                            start=(f == 0), stop=(f == _KF - 1))
                    yt = yp.tile([_P, _NB], dt.float32, name="yt", tag="yt")
                    nc.vector.tensor_scalar_add(
                        yt[:, :nb], py[:, :nb], bs[:, _KF + d:_KF + d + 1])
                    nc.sync.dma_start(
                        out=yT[d * _P:(d + 1) * _P, s0:s0 + nb],
                        in_=yt[:, :nb])
    nc.finalize()
    return nc


def kernel(x, expert_indices, w1, b1, w2, b2):
    global LAST_RESULT
    _ensure_path()
    _ensure_axon_hooks()
    import ml_dtypes
    from concourse.bass_utils import run_bass_kernel_spmd

    bf16 = ml_dtypes.bfloat16
    x = np.asarray(x)
    idxs = np.asarray(expert_indices)
    w1 = np.asarray(w1, dtype=np.float32)
    b1 = np.asarray(b1, dtype=np.float32)
    w2 = np.asarray(w2, dtype=np.float32)
    b2 = np.asarray(b2, dtype=np.float32)

    B, S, D = x.shape
    T = B * S
    E = w1.shape[0]
    K = idxs.shape[-1]
    assert D == _D and w1.shape[2] == _F and E == _NCORES

    xf = np.ascontiguousarray(x.reshape(T, D).astype(np.float32))
    idx = idxs.reshape(T, K)

    # Per-expert token lists, kept per top-k slot so every fancy-index
    # scatter below has unique indices (duplicates only occur when one
    # token picks the same expert in both slots -> two separate rows).
    slot_toks = [[np.nonzero(idx[:, k] == e)[0] for k in range(K)]
                 for e in range(E)]

    # Split each expert's rows into passes of <= _CAP rows so the
    # per-core SBUF-resident activation tile stays bounded no matter
    # how skewed the routing is. Uniform routing (the reference) stays
    # a single pass. Pieces are slices of one slot's list -> indices
    # stay unique within each piece.
    _CAP = 2048
    exp_pieces = []
    for e in range(E):
        pieces = []
        for sl in slot_toks[e]:
            for o in range(0, len(sl), _CAP):
                pieces.append(sl[o:o + _CAP])
        exp_pieces.append(pieces)

    passes = []
    cursors = [0] * E       # piece index per expert
    offs = [0] * E          # offset inside current piece
    while True:
        plan = []           # per expert: list of token arrays this pass
        any_rows = False
        for e in range(E):
            take, room = [], _CAP
            while room > 0 and cursors[e] < len(exp_pieces[e]):
                pc = exp_pieces[e][cursors[e]]
                part = pc[offs[e]: offs[e] + room]
                if len(part):
                    take.append(part)
                    room -= len(part)
                    offs[e] += len(part)
                if offs[e] >= len(pc):
                    cursors[e] += 1
                    offs[e] = 0
            if take:
                any_rows = True
            plan.append(take)
        if not any_rows:
            break
        passes.append(plan)
    if not passes:
        passes = [[[] for _ in range(E)]]

    wmaps = []
    for e in range(E):
        wmaps.append({
            "w1": np.ascontiguousarray(w1[e]).astype(bf16),
            "w2": np.ascontiguousarray(w2[e]).astype(bf16),
            "bias": np.ascontiguousarray(np.concatenate(
                [b1[e].reshape(_KF, _P).T, b2[e].reshape(_KD, _P).T],
                axis=1)).astype(np.float32),
        })

    out = np.zeros((T, D), dtype=np.float32)
    for plan in passes:
        counts = [int(sum(len(p) for p in plan[e])) for e in range(E)]
        C = max(max(counts), 64)
        C = ((C + 7) // 8) * 8

        in_maps = []
        for e in range(E):
            xTe = np.zeros((D, C), dtype=bf16)
            if counts[e]:
                toks = np.concatenate(plan[e])
                xTe[:, :counts[e]] = xf[toks].T.astype(bf16)
            in_maps.append({"xT": xTe, **wmaps[e]})

        nc = _cache.get(C)
        if nc is None:
            nc = _build(C)
            _cache[C] = nc

        res = run_bass_kernel_spmd(nc, in_maps, core_ids=list(range(_NCORES)))
        LAST_RESULT = res

        for e in range(E):
            yTe = np.asarray(res.results[e]["yT"])
            o = 0
            for piece in plan[e]:
                n = len(piece)
                if n:
                    out[piece] += yTe[:, o:o + n].T
                o += n
    return out.reshape(B, S, D)
